# revision 26
# baseline (speedup 1.0000x reference)
"""Masked self-attention Trainium2 Bass kernel.

Reference computation (per batch b):
    q = x @ Wq + bq ; k = x @ Wk + bk ; v = x @ Wv + bv      # [S, A]
    scores = (q @ k.T) / sqrt(S)  with causal mask            # [S, S]
    out = softmax(scores, axis=-1) @ v                        # [S, A]

Sharding: data-parallel over batch across 8 NeuronCores (B=32 -> 4 per core),
weights replicated. No collectives.

Per-core design. Mixed precision: the q/k projections and the score matmul
run in fp8e4m3 with DoubleRow perf mode (2 fp8 weights per PE cell ->
2 contraction rows per cycle, ~2x matmul throughput); the v projection and
the PV matmul stay bf16 (v-path errors hit the output directly; score-path
errors are damped by softmax). Accumulation is always fp32 in PSUM.
Rel-err budget: q/k fp8 ~2.6% -> score err ~0.01 abs -> output ~5e-3 rel.

  Stage A: DMA x[b] [S,E] fp32 (split across DMA queues; weights ride the
           SWDGE queue); fp32r PE-transpose via identity -> PSUM (1.5
           cyc/row), 4 transposes per PSUM tile; DVE copies each group into
           xT_bf [128, E/128, S] bf16, ACT copies into xT_f8 fp8 (strided
           3D-AP writes; the [P, n_e, s] layout doubles as the DoubleRow
           subtile-pair operand layout).
  Stage B: qT/kT [A,S] in 3D layout [128, A/128, S] fp8: 4 DoubleRow
           matmuls (contraction 2x128 per step) per 512-wide chunk; weights
           pre-scaled by 16 on-device so fp8 stays in normal range (Wq
           values ~0.02 < fp8 min normal 2^-6), un-scaled by the ACT
           PSUM->SBUF bias-copy (scale=1/16, fp8 out). v = xT_bf.T @ Wv
           bf16 -> [S,A] (no bias: softmax rows sum to 1, bv is added to
           the final output). v tiles carry 2 ones-columns for row-sums.
  Stage C: scoresT[k,q] per k-tile: 2 DoubleRow matmuls (m-pairs over A);
           causal-trimmed even chunks widened to >=256 moving cols;
           additive -1e9 mask on the diagonal block in PSUM (DVE); exp on
           ACT with scale=1/sqrt(S) folded in (PSUM->SBUF, bf16 out).
  Stage D: interleaved with C per tile: out_psum = sum_t expT[t].T @
           v_bf[t] (bf16) in two column chunks; ones-columns yield softmax
           row-sums; DVE reciprocal; DVE tensor_scalar scales rows by
           1/sum; GPSIMD adds broadcast bv; DMA out per 256-column half.
"""

import numpy as np
from contextlib import ExitStack

import concourse.bass as bass
import concourse.mybir as mybir
import concourse.tile as tile
from concourse import bacc
from concourse.bass_utils import run_bass_kernel_spmd
from concourse.masks import make_identity

P = 128
F32 = mybir.dt.float32
F32R = mybir.dt.float32r
BF16 = mybir.dt.bfloat16
FP8 = mybir.dt.float8e4
AF = mybir.ActivationFunctionType
DR = mybir.MatmulPerfMode.DoubleRow

N_CORES = 8
B, S, E, A = 32, 1000, 1024, 512
MASK_NEG = -1.0e9
WSCALE = 16.0  # fp8 weight pre-scale: Wq/Wk ~0.02 are below fp8 min-normal
SPAD = 1024  # padded free-dim stride of the [P, n, S] 3D tiles (16-aligned)


def _even_chunks(start, total, maxc):
    """Split [start, start+total) into ceil(total/maxc) near-even chunks,
    each of even size (required by fp32r matmul moving dim)."""
    assert total % 2 == 0
    n = max(1, -(-total // maxc))
    bounds = [start + ((i * total) // n) // 2 * 2 for i in range(n)]
    bounds.append(start + total)
    return [(bounds[i], bounds[i + 1] - bounds[i]) for i in range(n)]


def build(b_pc, s, e, a, reps=1):
    assert e % P == 0 and a % P == 0
    n_s = -(-s // P)
    n_e = e // P
    n_a = a // P
    inv_den = float(s) ** -0.5
    s_tiles = [(t * P, min(P, s - t * P)) for t in range(n_s)]
    h = a // 2  # PV column split: [0,h) and [h, a+2)

    nc = bacc.Bacc("TRN2")
    x = nc.dram_tensor("x", [b_pc, s, e], F32R, kind="ExternalInput").ap()
    w_dram = {
        "q": nc.dram_tensor("Wq", [e, a], F32, kind="ExternalInput").ap(),
        "k": nc.dram_tensor("Wk", [e, a], F32, kind="ExternalInput").ap(),
        "v": nc.dram_tensor("Wv", [e, a], F32, kind="ExternalInput").ap(),
    }
    b_dram = {
        "q": nc.dram_tensor("bq", [a], F32, kind="ExternalInput").ap(),
        "k": nc.dram_tensor("bk", [a], F32, kind="ExternalInput").ap(),
        "v": nc.dram_tensor("bv", [a], F32, kind="ExternalInput").ap(),
    }
    out = nc.dram_tensor("out", [b_pc, s, a], F32, kind="ExternalOutput").ap()

    with tile.TileContext(nc) as tc, ExitStack() as ctx:
        pool = ctx.enter_context(tc.tile_pool(name="sb", bufs=1))
        pp_tp = ctx.enter_context(tc.tile_pool(name="pp_tp", bufs=2, space="PSUM"))
        pp_proj = ctx.enter_context(tc.tile_pool(name="pp_proj", bufs=2, space="PSUM"))
        pp_score = ctx.enter_context(tc.tile_pool(name="pp_sc", bufs=2, space="PSUM"))
        pp_o1 = ctx.enter_context(tc.tile_pool(name="pp_o1", bufs=1, space="PSUM"))
        pp_o2 = ctx.enter_context(tc.tile_pool(name="pp_o2", bufs=1, space="PSUM"))

        # ---------------- constants ----------------
        ident_st = pool.tile([P, P], F32)
        make_identity(nc, ident_st)
        ident = pool.tile([P, P], F32R)
        nc.scalar.copy(ident[:], ident_st[:])

        # additive causal mask for the diagonal block:
        # keep 0 where col q >= row k (i.e. (y - x) >= 0), else fill -1e9
        amask = pool.tile([P, P], F32)
        nc.gpsimd.memset(amask, 0.0)
        nc.gpsimd.affine_select(
            out=amask, in_=amask,
            compare_op=mybir.AluOpType.is_ge,
            fill=MASK_NEG, base=0,
            pattern=[[1, P]], channel_multiplier=-1,
        )

        ones_stage = pool.tile([P, 2], BF16)
        nc.gpsimd.memset(ones_stage, 1.0)
        ones_f8 = pool.tile([P, 2], FP8)
        nc.gpsimd.memset(ones_f8, 1.0)

        # ---------------- weights / biases ----------------
        # q/k: fp8, pre-scaled by WSCALE, [P, n_e, a] DoubleRow layout, plus
        # unscaled bf16 copies for the accurate diagonal-block path.
        # v: bf16.
        w_sb = {}
        w_bf = {}
        for nm in ("q", "k", "v"):
            dst = pool.tile([P, n_e, a], FP8, tag=f"w_{nm}", bufs=1,
                            name=f"w8_{nm}")
            wb = pool.tile([P, n_e, a], BF16, tag=f"wb_{nm}", bufs=1,
                           name=f"wb_{nm}")
            w_bf[nm] = wb
            for u in range(n_e):
                w_stage = pool.tile([P, a], F32, tag="w_stage", bufs=2)
                nc.gpsimd.dma_start(w_stage[:], w_dram[nm][u * P:(u + 1) * P, :])
                nc.vector.tensor_scalar_mul(dst[:, u], w_stage[:], WSCALE)
                nc.vector.tensor_copy(wb[:, u], w_stage[:])
            w_sb[nm] = dst

        bias_sb = {}
        for nm in ("q", "k"):
            b_st = pool.tile([P, n_a], F32, tag=f"b_{nm}", bufs=1)
            nc.gpsimd.dma_start(
                b_st[:], b_dram[nm].rearrange("(m p) -> p m", p=P)
            )
            bias_sb[nm] = b_st

        bv_stage = pool.tile([1, a], F32)
        nc.gpsimd.dma_start(bv_stage[:], b_dram["v"][:])
        bv_bc = pool.tile([P, a], F32)
        nc.gpsimd.partition_broadcast(bv_bc[:], bv_stage[:])

        # ---------------- per-batch pipeline ----------------
        # reps>1 wraps the whole pipeline in an on-device loop re-running the
        # same work — used only to measure device exec time (amortizes the
        # per-dispatch RPC overhead, which otherwise hides the kernel).
        rep_ctx = (tc.For_i(0, reps, 1, hint_engines=tuple(nc.engines),
                            staggered_reset=True)
                   if reps > 1 else None)
        if rep_ctx is not None:
            ctx.enter_context(rep_ctx)
        for b in range(b_pc):
            # ---- stage A: load x, transpose to xT [E, S]: fp8 everywhere,
            # plus a bf16 copy of the first 128 columns only (the accurate
            # early-row path is their sole consumer) ----
            xT_bf = pool.tile([P, n_e, P], BF16, tag="xT_bf", bufs=2,
                              name=f"xTb{b}")
            xT_f8 = pool.tile([P, n_e, SPAD], FP8, tag="xT_f8", bufs=2,
                              name=f"xT8{b}")
            g = 4 if n_e % 4 == 0 else (2 if n_e % 2 == 0 else 1)
            for (s0, sl) in s_tiles:
                x_sb = pool.tile([P, e], F32R, tag="x", bufs=3)
                # split the load across DMA queues for parallelism; finer
                # split for the first batch, whose loads pace the pipeline fill
                nsp = 4 if b == 0 else 2
                w_sp = e // nsp
                for qi in range(nsp):
                    nc.sync.dma_start(
                        x_sb[:sl, qi * w_sp:(qi + 1) * w_sp],
                        x[b, s0:s0 + sl, qi * w_sp:(qi + 1) * w_sp],
                    )
                for u0 in range(0, n_e, g):
                    # g transposes share one PSUM tile; one strided 3D-AP
                    # copy per dtype moves all of them to SBUF
                    tp = pp_tp.tile([P, g * P], F32R, tag="tp")
                    for j in range(g):
                        nc.tensor.transpose(
                            tp[:, j * P:j * P + sl],
                            x_sb[:sl, (u0 + j) * P:(u0 + j + 1) * P],
                            ident[:sl, :sl],
                        )
                    tp3 = tp.rearrange("p (j c) -> p j c", c=P)
                    nc.vector.tensor_copy(
                        xT_f8[:, u0:u0 + g, s0:s0 + sl], tp3[:, :, :sl]
                    )
                    if s0 == 0:
                        nc.scalar.copy(
                            xT_bf[:, u0:u0 + g, 0:sl], tp3[:, :, :sl]
                        )

            # ---- stage B: projections ----
            # qT/kT [A, S] fp8 in DoubleRow 3D layout [P, n_a, SPAD]
            qkT = {}
            for nm in ("q", "k"):
                dest = pool.tile([P, n_a, SPAD], FP8, tag=f"{nm}T", bufs=2,
                                 name=f"{nm}T{b}")
                qkT[nm] = dest
                for m in range(n_a):
                    for (c0, cl) in _even_chunks(0, s, 512):
                        mm = pp_proj.tile([P, 512], F32, tag="proj")
                        for j in range(n_e // 2):
                            nc.tensor.matmul(
                                mm[:, :cl],
                                w_sb[nm][:, 2 * j:2 * j + 2, m * P:(m + 1) * P],
                                xT_f8[:, 2 * j:2 * j + 2, c0:c0 + cl],
                                start=(j == 0), stop=(j == n_e // 2 - 1),
                                perf_mode=DR,
                            )
                        nc.scalar.activation(
                            dest[:, m, c0:c0 + cl], mm[:, :cl], AF.Identity,
                            bias=bias_sb[nm][:, m:m + 1], scale=1.0 / WSCALE,
                        )

            # accurate bf16 q/k for the first s-tile's rows: the max fp8
            # error concentrates in early (short-softmax) rows, and for
            # q<128 the whole causal row lives in the 128x128 diagonal
            # block, so a bf16 diagonal path caps the error at bf16 level.
            qkT0 = {}
            for nm in ("q", "k"):
                dest0 = pool.tile([P, n_a, P], BF16, tag=f"{nm}T0", bufs=2,
                                  name=f"{nm}T0_{b}")
                qkT0[nm] = dest0
                for m in range(n_a):
                    mm0 = pp_proj.tile([P, 512], F32, tag="proj")
                    for u in range(n_e):
                        nc.tensor.matmul(
                            mm0[:, :P],
                            w_bf[nm][:, u, m * P:(m + 1) * P],
                            xT_bf[:, u, 0:P],
                            start=(u == 0), stop=(u == n_e - 1),
                        )
                    nc.vector.tensor_scalar_add(
                        dest0[:, m, :], mm0[:, :P], bias_sb[nm][:, m:m + 1])

            # v [S, A+2] natural layout; last two columns are ones (for the
            # softmax row-sums via the PV matmul). fp8 DoubleRow projection
            # in [P, n_s, VPAD] subtile layout (the t-dim doubles as the
            # DoubleRow k-pair dim for the PV matmul); s-tile 0 additionally
            # gets a bf16 copy for the accurate early-row PV.
            VPAD = 16 * (-(-(a + 2) // 16))
            v8_all = pool.tile([P, n_s, VPAD], FP8, tag="v8", bufs=2,
                               name=f"v8_{b}")
            # zero the garbage tail rows of the ragged last s-tile BEFORE the
            # projection writes (partition offset must be 32-aligned; the
            # overlap rows are rewritten by the ACT copy afterwards) so the
            # DoubleRow PV pairs contract clean zeros there
            sl_last = s_tiles[-1][1]
            zp = (sl_last // 32) * 32
            if sl_last < P:
                nc.gpsimd.memset(v8_all[zp:, n_s - 1, :], 0.0)
            for t, (s0, sl) in enumerate(s_tiles):
                vm = pp_proj.tile([P, 512], F32, tag="proj")
                for j in range(n_e // 2):
                    nc.tensor.matmul(
                        vm[:sl, :a],
                        xT_f8[:, 2 * j:2 * j + 2, s0:s0 + sl],
                        w_sb["v"][:, 2 * j:2 * j + 2, :],
                        start=(j == 0), stop=(j == n_e // 2 - 1),
                        perf_mode=DR,
                    )
                nc.scalar.activation(
                    v8_all[:sl, t, 0:a], vm[:sl, :a], AF.Identity,
                    scale=1.0 / WSCALE,
                )
                nc.scalar.copy(v8_all[:sl, t, a:a + 2], ones_f8[:sl, :])
            # bf16 v for s-tile 0 (the accurate early-row path)
            v0_bf = pool.tile([P, a + 2], BF16, tag="v0", bufs=2,
                              name=f"v0_{b}")
            vm0 = pp_proj.tile([P, 512], F32, tag="proj")
            for u in range(n_e):
                nc.tensor.matmul(
                    vm0[:P, :a], xT_bf[:, u, 0:P], w_bf["v"][:, u],
                    start=(u == 0), stop=(u == n_e - 1),
                )
            nc.vector.tensor_copy(v0_bf[:, :a], vm0[:P, :a])
            nc.scalar.copy(v0_bf[:, a:a + 2], ones_stage[:, :])

            # ---- stages C+D interleaved per tile: scoresT/exp for k-tile
            # t, then PV/out for q-tile t (its expT deps are all ready) ----
            # exp(scoresT) in fp8, stored at absolute q-columns in a
            # [P, n_s, SPAD] subtile layout (so PV can contract k-tile
            # PAIRS with DoubleRow). et0_bf holds the accurate bf16
            # diagonal block for q<128.
            expT = pool.tile([P, n_s, SPAD], FP8, tag="expT", bufs=2,
                             name=f"et{b}")
            et0_bf = pool.tile([P, P], BF16, tag="et0", bufs=2,
                               name=f"et0_{b}")
            # zero the strips that DoubleRow PV pairs read beyond a tile's
            # causal start (odd tiles t at q-cols [k0-P, k0)), and the
            # ragged last tile's unwritten rows
            for t in range(3, n_s, 2):
                nc.gpsimd.memset(expT[:, t, t * P - P:t * P], 0.0)
            if sl_last < P:
                nc.gpsimd.memset(expT[zp:, n_s - 1, :], 0.0)
            for t, (k0, kl) in enumerate(s_tiles):
                if t == 0:
                    # accurate bf16 diagonal block for q-cols [0, P)
                    sc0 = pp_score.tile([P, 512], F32, tag="score")
                    for m in range(n_a):
                        nc.tensor.matmul(
                            sc0[:kl, :P],
                            qkT0["k"][:, m, :kl], qkT0["q"][:, m, :],
                            start=(m == 0), stop=(m == n_a - 1),
                        )
                    nc.vector.tensor_add(
                        sc0[:kl, :kl], sc0[:kl, :kl], amask[:kl, :kl])
                    nc.scalar.activation(
                        et0_bf[:kl, :], sc0[:kl, :P], AF.Exp, scale=inv_den)
                    chunks = _even_chunks(k0 + P, s - k0 - P, 512)
                    diag_done = True
                else:
                    chunks = _even_chunks(k0, s - k0, 512)
                    diag_done = False
                for pi, (c0, cl) in enumerate(chunks):
                    # widen narrow chunks leftward into the masked
                    # (never-read) region to keep the moving dim >=256;
                    # exp/mask then read at offset `ext`.
                    ext = min(256 - cl, c0) if cl < 256 else 0
                    sc = pp_score.tile([P, 512], F32, tag="score")
                    for mj in range(n_a // 2):
                        nc.tensor.matmul(
                            sc[:kl, :ext + cl],
                            qkT["k"][:, 2 * mj:2 * mj + 2, k0:k0 + kl],
                            qkT["q"][:, 2 * mj:2 * mj + 2, c0 - ext:c0 + cl],
                            start=(mj == 0), stop=(mj == n_a // 2 - 1),
                            perf_mode=DR,
                        )
                    if pi == 0 and not diag_done:
                        # diagonal block: additive causal mask in PSUM
                        nc.vector.tensor_add(
                            sc[:kl, ext:ext + kl], sc[:kl, ext:ext + kl],
                            amask[:kl, :kl]
                        )
                    nc.scalar.activation(
                        expT[:kl, t, c0:c0 + cl],
                        sc[:kl, ext:ext + cl], AF.Exp, scale=inv_den,
                    )

                i, (q0, il) = t, s_tiles[t]
                op1 = pp_o1.tile([P, h], F32, tag="op1")
                op2 = pp_o2.tile([P, a - h + 2], F32, tag="op2")
                if i == 0:
                    # accurate bf16 PV for the early rows
                    nc.tensor.matmul(op1[:il, :], et0_bf[:, :il],
                                     v0_bf[:, 0:h], start=True, stop=True)
                    nc.tensor.matmul(op2[:il, :], et0_bf[:, :il],
                                     v0_bf[:, h:a + 2], start=True, stop=True)
                else:
                    # fp8 DoubleRow PV over k-tile pairs
                    pairs = list(range(0, i + 1, 2))
                    for tj in pairs:
                        lhs = expT[:, tj:tj + 2, q0:q0 + il]
                        st = tj == 0
                        sp = tj == pairs[-1]
                        nc.tensor.matmul(
                            op1[:il, :], lhs, v8_all[:, tj:tj + 2, 0:h],
                            start=st, stop=sp, perf_mode=DR,
                        )
                        nc.tensor.matmul(
                            op2[:il, :], lhs, v8_all[:, tj:tj + 2, h:a + 2],
                            start=st, stop=sp, perf_mode=DR,
                        )
                rec = pool.tile([P, 1], F32, tag="rec", bufs=2)
                nc.vector.reciprocal(rec[:il, :], op2[:il, a - h:a - h + 1])
                o_sb = pool.tile([P, a], F32, tag="o_sb", bufs=3)
                # epilogue split per half so scale+bias/store pipeline;
                # (psum * 1/rowsum) + bv fused in one DVE op — gpsimd
                # shares the SBUF port with DVE, so a separate bv-add
                # there costs combined DVE+Pool port time
                nc.vector.scalar_tensor_tensor(
                    o_sb[:il, 0:h], op1[:il, :], rec[:il, 0:1],
                    bv_bc[:il, 0:h],
                    op0=mybir.AluOpType.mult, op1=mybir.AluOpType.add)
                nc.sync.dma_start(out[b, q0:q0 + il, 0:h], o_sb[:il, 0:h])
                nc.vector.scalar_tensor_tensor(
                    o_sb[:il, h:a], op2[:il, 0:a - h], rec[:il, 0:1],
                    bv_bc[:il, h:a],
                    op0=mybir.AluOpType.mult, op1=mybir.AluOpType.add)
                nc.sync.dma_start(out[b, q0:q0 + il, h:a], o_sb[:il, h:a])

    nc.compile()
    return nc


_BUILT = {}


def _get_nc(b_pc, s, e, a):
    key = (b_pc, s, e, a)
    if key not in _BUILT:
        _BUILT[key] = build(b_pc, s, e, a)
    return _BUILT[key]


def run_sharded(inputs, b_pc, s, e, a, **run_kwargs):
    """Run the SPMD kernel over N_CORES cores, sharding batch dim of x."""
    x = np.ascontiguousarray(inputs["x"], dtype=np.float32)
    b_total = x.shape[0]
    assert b_total == b_pc * N_CORES
    shared = {
        "Wq": np.ascontiguousarray(inputs["Wq"], dtype=np.float32),
        "Wk": np.ascontiguousarray(inputs["Wk"], dtype=np.float32),
        "Wv": np.ascontiguousarray(inputs["Wv"], dtype=np.float32),
        "bq": np.ascontiguousarray(inputs["bq"], dtype=np.float32),
        "bk": np.ascontiguousarray(inputs["bk"], dtype=np.float32),
        "bv": np.ascontiguousarray(inputs["bv"], dtype=np.float32),
    }
    in_maps = [
        {"x": x[c * b_pc:(c + 1) * b_pc], **shared} for c in range(N_CORES)
    ]
    nc = _get_nc(b_pc, s, e, a)
    res = run_bass_kernel_spmd(nc, in_maps, core_ids=list(range(N_CORES)),
                               **run_kwargs)
    full = np.concatenate([res.results[c]["out"] for c in range(N_CORES)], axis=0)
    return full, res


def kernel(**inputs) -> np.ndarray:
    out, _ = run_sharded(inputs, B // N_CORES, S, E, A)
    return out


# revision 27
# speedup vs baseline: 1.3954x; 1.3954x over previous
"""Masked self-attention Trainium2 Bass kernel.

Reference computation (per batch b):
    q = x @ Wq + bq ; k = x @ Wk + bk ; v = x @ Wv + bv      # [S, A]
    scores = (q @ k.T) / sqrt(S)  with causal mask            # [S, S]
    out = softmax(scores, axis=-1) @ v                        # [S, A]

Sharding: data-parallel over batch across 8 NeuronCores (B=32 -> 4 per core),
weights replicated. No collectives.

Per-core design. Mixed precision: the q/k projections and the score matmul
run in fp8e4m3 with DoubleRow perf mode (2 fp8 weights per PE cell ->
2 contraction rows per cycle, ~2x matmul throughput); the v projection and
the PV matmul stay bf16 (v-path errors hit the output directly; score-path
errors are damped by softmax). Accumulation is always fp32 in PSUM.
Rel-err budget: q/k fp8 ~2.6% -> score err ~0.01 abs -> output ~5e-3 rel.

  Stage A: DMA x[b] [S,E] fp32 (split across DMA queues; weights ride the
           SWDGE queue); fp32r PE-transpose via identity -> PSUM (1.5
           cyc/row), 4 transposes per PSUM tile; DVE copies each group into
           xT_bf [128, E/128, S] bf16, ACT copies into xT_f8 fp8 (strided
           3D-AP writes; the [P, n_e, s] layout doubles as the DoubleRow
           subtile-pair operand layout).
  Stage B: qT/kT [A,S] in 3D layout [128, A/128, S] fp8: 4 DoubleRow
           matmuls (contraction 2x128 per step) per 512-wide chunk; weights
           pre-scaled by 16 on-device so fp8 stays in normal range (Wq
           values ~0.02 < fp8 min normal 2^-6), un-scaled by the ACT
           PSUM->SBUF bias-copy (scale=1/16, fp8 out). v = xT_bf.T @ Wv
           bf16 -> [S,A] (no bias: softmax rows sum to 1, bv is added to
           the final output). v tiles carry 2 ones-columns for row-sums.
  Stage C: scoresT[k,q] per k-tile: 2 DoubleRow matmuls (m-pairs over A);
           causal-trimmed even chunks widened to >=256 moving cols;
           additive -1e9 mask on the diagonal block in PSUM (DVE); exp on
           ACT with scale=1/sqrt(S) folded in (PSUM->SBUF, bf16 out).
  Stage D: interleaved with C per tile: out_psum = sum_t expT[t].T @
           v_bf[t] (bf16) in two column chunks; ones-columns yield softmax
           row-sums; DVE reciprocal; DVE tensor_scalar scales rows by
           1/sum; GPSIMD adds broadcast bv; DMA out per 256-column half.
"""

import numpy as np
from contextlib import ExitStack

import concourse.bass as bass
import concourse.mybir as mybir
import concourse.tile as tile
from concourse import bacc
from concourse.bass_utils import run_bass_kernel_spmd
from concourse.masks import make_identity

P = 128
F32 = mybir.dt.float32
F32R = mybir.dt.float32r
BF16 = mybir.dt.bfloat16
FP8 = mybir.dt.float8e4
AF = mybir.ActivationFunctionType
DR = mybir.MatmulPerfMode.DoubleRow

N_CORES = 8
B, S, E, A = 32, 1000, 1024, 512
MASK_NEG = -1.0e9
WSCALE = 16.0  # fp8 weight pre-scale: Wq/Wk ~0.02 are below fp8 min-normal
SPAD = 1024  # padded free-dim stride of the [P, n, S] 3D tiles (16-aligned)


def _even_chunks(start, total, maxc):
    """Split [start, start+total) into ceil(total/maxc) near-even chunks,
    each of even size (required by fp32r matmul moving dim)."""
    assert total % 2 == 0
    n = max(1, -(-total // maxc))
    bounds = [start + ((i * total) // n) // 2 * 2 for i in range(n)]
    bounds.append(start + total)
    return [(bounds[i], bounds[i + 1] - bounds[i]) for i in range(n)]


def build(b_pc, s, e, a, reps=1):
    assert e % P == 0 and a % P == 0
    n_s = -(-s // P)
    n_e = e // P
    n_a = a // P
    inv_den = float(s) ** -0.5
    s_tiles = [(t * P, min(P, s - t * P)) for t in range(n_s)]
    h = a // 2  # PV column split: [0,h) and [h, a+2)

    nc = bacc.Bacc("TRN2")
    x = nc.dram_tensor("x", [b_pc, s, e], F32R, kind="ExternalInput").ap()
    w_dram = {
        "q": nc.dram_tensor("Wq", [e, a], F32, kind="ExternalInput").ap(),
        "k": nc.dram_tensor("Wk", [e, a], F32, kind="ExternalInput").ap(),
        "v": nc.dram_tensor("Wv", [e, a], F32, kind="ExternalInput").ap(),
    }
    b_dram = {
        "q": nc.dram_tensor("bq", [a], F32, kind="ExternalInput").ap(),
        "k": nc.dram_tensor("bk", [a], F32, kind="ExternalInput").ap(),
        "v": nc.dram_tensor("bv", [a], F32, kind="ExternalInput").ap(),
    }
    out = nc.dram_tensor("out", [b_pc, s, a], F32, kind="ExternalOutput").ap()

    with tile.TileContext(nc) as tc, ExitStack() as ctx:
        pool = ctx.enter_context(tc.tile_pool(name="sb", bufs=1))
        pp_tp = ctx.enter_context(tc.tile_pool(name="pp_tp", bufs=2, space="PSUM"))
        pp_proj = ctx.enter_context(tc.tile_pool(name="pp_proj", bufs=2, space="PSUM"))
        pp_score = ctx.enter_context(tc.tile_pool(name="pp_sc", bufs=2, space="PSUM"))
        pp_o1 = ctx.enter_context(tc.tile_pool(name="pp_o1", bufs=1, space="PSUM"))
        pp_o2 = ctx.enter_context(tc.tile_pool(name="pp_o2", bufs=1, space="PSUM"))

        # ---------------- constants ----------------
        ident_st = pool.tile([P, P], F32)
        make_identity(nc, ident_st)
        ident = pool.tile([P, P], F32R)
        nc.scalar.copy(ident[:], ident_st[:])

        # additive causal mask for the diagonal block:
        # keep 0 where col q >= row k (i.e. (y - x) >= 0), else fill -1e9
        amask = pool.tile([P, P], F32)
        nc.gpsimd.memset(amask, 0.0)
        nc.gpsimd.affine_select(
            out=amask, in_=amask,
            compare_op=mybir.AluOpType.is_ge,
            fill=MASK_NEG, base=0,
            pattern=[[1, P]], channel_multiplier=-1,
        )

        ones_stage = pool.tile([P, 2], BF16)
        nc.gpsimd.memset(ones_stage, 1.0)
        ones_f8 = pool.tile([P, 2], FP8)
        nc.gpsimd.memset(ones_f8, 1.0)

        # ---------------- weights / biases ----------------
        # q/k: fp8, pre-scaled by WSCALE, [P, n_e, a] DoubleRow layout, plus
        # unscaled bf16 copies for the accurate diagonal-block path.
        # v: bf16.
        w_sb = {}
        w_bf = {}
        for nm in ("q", "k", "v"):
            dst = pool.tile([P, n_e, a], FP8, tag=f"w_{nm}", bufs=1,
                            name=f"w8_{nm}")
            wb = pool.tile([P, n_e, a], BF16, tag=f"wb_{nm}", bufs=1,
                           name=f"wb_{nm}")
            w_bf[nm] = wb
            for u in range(n_e):
                w_stage = pool.tile([P, a], F32, tag="w_stage", bufs=2)
                nc.gpsimd.dma_start(w_stage[:], w_dram[nm][u * P:(u + 1) * P, :])
                nc.vector.tensor_scalar_mul(dst[:, u], w_stage[:], WSCALE)
                nc.vector.tensor_copy(wb[:, u], w_stage[:])
            w_sb[nm] = dst

        bias_sb = {}
        for nm in ("q", "k"):
            b_st = pool.tile([P, n_a], F32, tag=f"b_{nm}", bufs=1)
            nc.gpsimd.dma_start(
                b_st[:], b_dram[nm].rearrange("(m p) -> p m", p=P)
            )
            bias_sb[nm] = b_st

        bv_stage = pool.tile([1, a], F32)
        nc.gpsimd.dma_start(bv_stage[:], b_dram["v"][:])
        bv_bc = pool.tile([P, a], F32)
        nc.gpsimd.partition_broadcast(bv_bc[:], bv_stage[:])

        # ---------------- per-batch pipeline ----------------
        # reps>1 wraps the whole pipeline in an on-device loop re-running the
        # same work — used only to measure device exec time (amortizes the
        # per-dispatch RPC overhead, which otherwise hides the kernel).
        rep_ctx = (tc.For_i(0, reps, 1, hint_engines=tuple(nc.engines),
                            staggered_reset=True)
                   if reps > 1 else None)
        if rep_ctx is not None:
            ctx.enter_context(rep_ctx)
        for b in range(b_pc):
            # ---- stage A: load x, transpose to xT [E, S]: fp8 everywhere,
            # plus a bf16 copy of the first 128 columns only (the accurate
            # early-row path is their sole consumer) ----
            xT_bf = pool.tile([P, n_e, P], BF16, tag="xT_bf", bufs=2,
                              name=f"xTb{b}")
            xT_f8 = pool.tile([P, n_e, SPAD], FP8, tag="xT_f8", bufs=2,
                              name=f"xT8{b}")
            g = 4 if n_e % 4 == 0 else (2 if n_e % 2 == 0 else 1)
            for (s0, sl) in s_tiles:
                x_sb = pool.tile([P, e], F32R, tag="x", bufs=3)
                # split the load across DMA queues for parallelism; finer
                # split for the first batch, whose loads pace the pipeline fill
                nsp = 4 if b == 0 else 2
                w_sp = e // nsp
                for qi in range(nsp):
                    nc.sync.dma_start(
                        x_sb[:sl, qi * w_sp:(qi + 1) * w_sp],
                        x[b, s0:s0 + sl, qi * w_sp:(qi + 1) * w_sp],
                    )
                for u0 in range(0, n_e, g):
                    # g transposes share one PSUM tile; one strided 3D-AP
                    # copy per dtype moves all of them to SBUF
                    tp = pp_tp.tile([P, g * P], F32R, tag="tp")
                    for j in range(g):
                        nc.tensor.transpose(
                            tp[:, j * P:j * P + sl],
                            x_sb[:sl, (u0 + j) * P:(u0 + j + 1) * P],
                            ident[:sl, :sl],
                        )
                    tp3 = tp.rearrange("p (j c) -> p j c", c=P)
                    nc.vector.tensor_copy(
                        xT_f8[:, u0:u0 + g, s0:s0 + sl], tp3[:, :, :sl]
                    )
                    if s0 == 0:
                        nc.scalar.copy(
                            xT_bf[:, u0:u0 + g, 0:sl], tp3[:, :, :sl]
                        )

            # ---- stage B: projections ----
            # qT/kT [A, S] fp8 in DoubleRow 3D layout [P, n_a, SPAD]
            qkT = {}
            for nm in ("q", "k"):
                dest = pool.tile([P, n_a, SPAD], FP8, tag=f"{nm}T", bufs=2,
                                 name=f"{nm}T{b}")
                qkT[nm] = dest
                for m in range(n_a):
                    for (c0, cl) in _even_chunks(0, s, 512):
                        mm = pp_proj.tile([P, 512], F32, tag="proj")
                        for j in range(n_e // 2):
                            nc.tensor.matmul(
                                mm[:, :cl],
                                w_sb[nm][:, 2 * j:2 * j + 2, m * P:(m + 1) * P],
                                xT_f8[:, 2 * j:2 * j + 2, c0:c0 + cl],
                                start=(j == 0), stop=(j == n_e // 2 - 1),
                                perf_mode=DR,
                            )
                        nc.scalar.activation(
                            dest[:, m, c0:c0 + cl], mm[:, :cl], AF.Identity,
                            bias=bias_sb[nm][:, m:m + 1], scale=1.0 / WSCALE,
                        )

            # accurate bf16 q/k for the first s-tile's rows: the max fp8
            # error concentrates in early (short-softmax) rows, and for
            # q<128 the whole causal row lives in the 128x128 diagonal
            # block, so a bf16 diagonal path caps the error at bf16 level.
            qkT0 = {}
            for nm in ("q", "k"):
                dest0 = pool.tile([P, n_a, P], BF16, tag=f"{nm}T0", bufs=2,
                                  name=f"{nm}T0_{b}")
                qkT0[nm] = dest0
                for m in range(n_a):
                    mm0 = pp_proj.tile([P, 512], F32, tag="proj")
                    for u in range(n_e):
                        nc.tensor.matmul(
                            mm0[:, :P],
                            w_bf[nm][:, u, m * P:(m + 1) * P],
                            xT_bf[:, u, 0:P],
                            start=(u == 0), stop=(u == n_e - 1),
                        )
                    nc.vector.tensor_scalar_add(
                        dest0[:, m, :], mm0[:, :P], bias_sb[nm][:, m:m + 1])

            # v [S, A+2] natural layout; last two columns are ones (for the
            # softmax row-sums via the PV matmul). fp8 DoubleRow projection
            # in [P, n_s, VPAD] subtile layout (the t-dim doubles as the
            # DoubleRow k-pair dim for the PV matmul); s-tile 0 additionally
            # gets a bf16 copy for the accurate early-row PV.
            VPAD = 16 * (-(-(a + 2) // 16))
            v8_all = pool.tile([P, n_s, VPAD], FP8, tag="v8", bufs=2,
                               name=f"v8_{b}")
            # zero the garbage tail rows of the ragged last s-tile BEFORE the
            # projection writes (partition offset must be 32-aligned; the
            # overlap rows are rewritten by the ACT copy afterwards) so the
            # DoubleRow PV pairs contract clean zeros there
            sl_last = s_tiles[-1][1]
            zp = (sl_last // 32) * 32
            if sl_last < P:
                nc.gpsimd.memset(v8_all[zp:, n_s - 1, :], 0.0)
            for t, (s0, sl) in enumerate(s_tiles):
                vm = pp_proj.tile([P, 512], F32, tag="proj")
                for j in range(n_e // 2):
                    nc.tensor.matmul(
                        vm[:sl, :a],
                        xT_f8[:, 2 * j:2 * j + 2, s0:s0 + sl],
                        w_sb["v"][:, 2 * j:2 * j + 2, :],
                        start=(j == 0), stop=(j == n_e // 2 - 1),
                        perf_mode=DR,
                    )
                nc.scalar.activation(
                    v8_all[:sl, t, 0:a], vm[:sl, :a], AF.Identity,
                    scale=1.0 / WSCALE,
                )
                nc.scalar.copy(v8_all[:sl, t, a:a + 2], ones_f8[:sl, :])
            # bf16 v for s-tile 0 (the accurate early-row path)
            v0_bf = pool.tile([P, a + 2], BF16, tag="v0", bufs=2,
                              name=f"v0_{b}")
            vm0 = pp_proj.tile([P, 512], F32, tag="proj")
            for u in range(n_e):
                nc.tensor.matmul(
                    vm0[:P, :a], xT_bf[:, u, 0:P], w_bf["v"][:, u],
                    start=(u == 0), stop=(u == n_e - 1),
                )
            nc.vector.tensor_copy(v0_bf[:, :a], vm0[:P, :a])
            nc.scalar.copy(v0_bf[:, a:a + 2], ones_stage[:, :])

            # ---- stages C+D interleaved per tile: scoresT/exp for k-tile
            # t, then PV/out for q-tile t (its expT deps are all ready) ----
            # exp(scoresT) in fp8, stored at absolute q-columns in a
            # [P, n_s, SPAD] subtile layout (so PV can contract k-tile
            # PAIRS with DoubleRow). et0_bf holds the accurate bf16
            # diagonal block for q<128.
            expT = pool.tile([P, n_s, SPAD], FP8, tag="expT", bufs=2,
                             name=f"et{b}")
            et0_bf = pool.tile([P, P], BF16, tag="et0", bufs=2,
                               name=f"et0_{b}")
            # zero the strips that DoubleRow PV pairs read beyond a tile's
            # causal start (odd tiles t at q-cols [k0-P, k0)), and the
            # ragged last tile's unwritten rows
            for t in range(3, n_s, 2):
                nc.gpsimd.memset(expT[:, t, t * P - P:t * P], 0.0)
            if sl_last < P:
                nc.gpsimd.memset(expT[zp:, n_s - 1, :], 0.0)
            for t, (k0, kl) in enumerate(s_tiles):
                if t == 0:
                    # accurate bf16 diagonal block for q-cols [0, P)
                    sc0 = pp_score.tile([P, 512], F32, tag="score")
                    for m in range(n_a):
                        nc.tensor.matmul(
                            sc0[:kl, :P],
                            qkT0["k"][:, m, :kl], qkT0["q"][:, m, :],
                            start=(m == 0), stop=(m == n_a - 1),
                        )
                    nc.vector.tensor_add(
                        sc0[:kl, :kl], sc0[:kl, :kl], amask[:kl, :kl])
                    nc.scalar.activation(
                        et0_bf[:kl, :], sc0[:kl, :P], AF.Exp, scale=inv_den)
                    chunks = _even_chunks(k0 + P, s - k0 - P, 512)
                    diag_done = True
                else:
                    chunks = _even_chunks(k0, s - k0, 512)
                    diag_done = False
                for pi, (c0, cl) in enumerate(chunks):
                    # widen narrow chunks leftward into the masked
                    # (never-read) region to keep the moving dim >=256;
                    # exp/mask then read at offset `ext`.
                    ext = min(256 - cl, c0) if cl < 256 else 0
                    sc = pp_score.tile([P, 512], F32, tag="score")
                    for mj in range(n_a // 2):
                        nc.tensor.matmul(
                            sc[:kl, :ext + cl],
                            qkT["k"][:, 2 * mj:2 * mj + 2, k0:k0 + kl],
                            qkT["q"][:, 2 * mj:2 * mj + 2, c0 - ext:c0 + cl],
                            start=(mj == 0), stop=(mj == n_a // 2 - 1),
                            perf_mode=DR,
                        )
                    if pi == 0 and not diag_done:
                        # diagonal block: additive causal mask in PSUM
                        nc.vector.tensor_add(
                            sc[:kl, ext:ext + kl], sc[:kl, ext:ext + kl],
                            amask[:kl, :kl]
                        )
                    nc.scalar.activation(
                        expT[:kl, t, c0:c0 + cl],
                        sc[:kl, ext:ext + cl], AF.Exp, scale=inv_den,
                    )

                i, (q0, il) = t, s_tiles[t]
                op1 = pp_o1.tile([P, h], F32, tag="op1")
                op2 = pp_o2.tile([P, a - h + 2], F32, tag="op2")
                if i == 0:
                    # accurate bf16 PV for the early rows
                    nc.tensor.matmul(op1[:il, :], et0_bf[:, :il],
                                     v0_bf[:, 0:h], start=True, stop=True)
                    nc.tensor.matmul(op2[:il, :], et0_bf[:, :il],
                                     v0_bf[:, h:a + 2], start=True, stop=True)
                else:
                    # fp8 DoubleRow PV over k-tile pairs
                    pairs = list(range(0, i + 1, 2))
                    for tj in pairs:
                        lhs = expT[:, tj:tj + 2, q0:q0 + il]
                        st = tj == 0
                        sp = tj == pairs[-1]
                        nc.tensor.matmul(
                            op1[:il, :], lhs, v8_all[:, tj:tj + 2, 0:h],
                            start=st, stop=sp, perf_mode=DR,
                        )
                        nc.tensor.matmul(
                            op2[:il, :], lhs, v8_all[:, tj:tj + 2, h:a + 2],
                            start=st, stop=sp, perf_mode=DR,
                        )
                rec = pool.tile([P, 1], F32, tag="rec", bufs=2)
                nc.vector.reciprocal(rec[:il, :], op2[:il, a - h:a - h + 1])
                o_sb = pool.tile([P, a], F32, tag="o_sb", bufs=3)
                # epilogue split per half so scale/bias-add/store pipeline
                nc.vector.tensor_scalar_mul(
                    o_sb[:il, 0:h], op1[:il, :], rec[:il, 0:1])
                nc.gpsimd.tensor_add(
                    o_sb[:il, 0:h], o_sb[:il, 0:h], bv_bc[:il, 0:h])
                nc.sync.dma_start(out[b, q0:q0 + il, 0:h], o_sb[:il, 0:h])
                nc.vector.tensor_scalar_mul(
                    o_sb[:il, h:a], op2[:il, 0:a - h], rec[:il, 0:1])
                nc.gpsimd.tensor_add(
                    o_sb[:il, h:a], o_sb[:il, h:a], bv_bc[:il, h:a])
                nc.sync.dma_start(out[b, q0:q0 + il, h:a], o_sb[:il, h:a])

    nc.compile()
    return nc


_BUILT = {}


def _get_nc(b_pc, s, e, a):
    key = (b_pc, s, e, a)
    if key not in _BUILT:
        _BUILT[key] = build(b_pc, s, e, a)
    return _BUILT[key]


def run_sharded(inputs, b_pc, s, e, a, **run_kwargs):
    """Run the SPMD kernel over N_CORES cores, sharding batch dim of x."""
    x = np.ascontiguousarray(inputs["x"], dtype=np.float32)
    b_total = x.shape[0]
    assert b_total == b_pc * N_CORES
    shared = {
        "Wq": np.ascontiguousarray(inputs["Wq"], dtype=np.float32),
        "Wk": np.ascontiguousarray(inputs["Wk"], dtype=np.float32),
        "Wv": np.ascontiguousarray(inputs["Wv"], dtype=np.float32),
        "bq": np.ascontiguousarray(inputs["bq"], dtype=np.float32),
        "bk": np.ascontiguousarray(inputs["bk"], dtype=np.float32),
        "bv": np.ascontiguousarray(inputs["bv"], dtype=np.float32),
    }
    in_maps = [
        {"x": x[c * b_pc:(c + 1) * b_pc], **shared} for c in range(N_CORES)
    ]
    nc = _get_nc(b_pc, s, e, a)
    res = run_bass_kernel_spmd(nc, in_maps, core_ids=list(range(N_CORES)),
                               **run_kwargs)
    full = np.concatenate([res.results[c]["out"] for c in range(N_CORES)], axis=0)
    return full, res


def kernel(**inputs) -> np.ndarray:
    out, _ = run_sharded(inputs, B // N_CORES, S, E, A)
    return out
